# revision 40
# baseline (speedup 1.0000x reference)
"""GroupQueryAttention on 8 TRN2 NeuronCores.

Strategy: tensor-parallel over heads. H=32 query heads, KV=8 kv heads,
group size G=4 -> each core owns exactly 1 kv head and its 4 query heads.
Per core:
  - QKV projections from a replicated (pre-transposed, channels-major) input
  - RoPE on Q/K (rotate-half, done on DVE across partition halves)
  - attention with scores computed TRANSPOSED ([keys, q] layout) so the
    exp(scores) tiles feed the V-matmul directly as the moving operand;
    softmax normalization is deferred: O = V.E, then ctx = O * (1/colsum(E))
  - partial output ctx @ Wo_shard  (row-shard of Wo)
Host sums the 8 partial outputs (the "all-reduce" of the row-parallel Wo).

Perf structure (v2):
  - dual DMA queues: loads are split between the SP (sync) and Activation
    (scalar) hardware DGE queues -- the single-queue serialization of v1
    starved the PE at kernel start and backed up the output at the end
  - quarter-0 projections are emitted as a chunk-major WAVE across the six
    projection streams (K, V, Q0..Q3, weights interleaved per k-group in
    one "wall" tensor) so the PE consumes each arriving x chunk at ~1/6th
    the single-stream rate -- the cold-start DMA can keep up and the PE
    never idles (idle triggers a ~7us half-speed HAM window)
  - x chunks double-buffered (bufs=2) so next-quarter prefetch has no WAR
    dependency and can be issued early on either queue
  - softmax normalization: colsum is computed pre-broadcast by a single
    ones[128,128] matmul (same cost as the old ones-column matmul), then
    DVE reciprocal_approx_fast + DVE scale. No gpsimd hop (the
    partition_broadcast custom op had ~1.2us latency on the critical path)
  - out-projection PSUM allocations rotate across three pool tags so a
    block's matmuls never wait on drains queued behind the previous head's
    exp avalanche; tail blocks' output DMAs alternate queues
  - a warm-up chain of tiny matmuls (computing the exp bias constant)
    lifts the PE HAM clock gate during the initial DMA wait
"""

import itertools
import sys

sys.path.insert(0, "/opt/trn_rl_repo")

from contextlib import ExitStack

import numpy as np
import ml_dtypes

import concourse.bass as bass
import concourse.bacc as bacc
import concourse.tile as tile
from concourse import mybir
from concourse.bass_utils import run_bass_kernel_spmd

BF16 = ml_dtypes.bfloat16

S = 2048          # sequence length
DIN = 4096        # model dim
H, KV, DH = 32, 8, 128
G = H // KV       # 4 query heads per kv head
NCORES = 8
HPC = H // NCORES     # 4 query heads per core
DPC = HPC * DH        # 512 = per-core q-projection width

NQ = 4            # s-quarters (chunks of 512 queries)
QC = S // NQ      # 512
KT = 128          # key tile (partition dim of transposed scores)
NKT = S // KT     # 16 key tiles
NK = DIN // 128   # 32 contraction tiles for projections
NXC = 8           # x chunks per quarter (k-groups of CW)
CW = NK // NXC    # 4 k-tiles per x chunk
NS = 6            # projection streams: K, V, Q0..Q3
SCALE = 1.0 / float(np.sqrt(DH))
EXP_BIAS = -10.0  # constant shift inside exp; cancels in normalization
NWARM = 80


def build_nc():
    """Build the per-core Bass program (same program on all 8 cores; the
    per-core weight shards arrive via in_maps)."""
    nc = bacc.Bacc()
    dt = mybir.dt

    # ---- DRAM parameters (host-prepared layouts; all DMA-contiguous) ----
    # x[p, sq, k, sc] = x_orig[512*sq + sc, 128*k + p]   (channels-major)
    x = nc.declare_dram_parameter("x", [128, NQ, NK, QC], dt.bfloat16, isOutput=False)
    # wall[p, k, s, d]: s=0 -> Wk_shard, s=1 -> Wv_shard, s=2+h -> Wq head h
    wall = nc.declare_dram_parameter("wall", [128, NK, NS, DH], dt.bfloat16,
                                     isOutput=False)
    # wo[p, h, n] = Wo_shard[128*h + p, n]
    wo = nc.declare_dram_parameter("wo", [128, HPC, DIN], dt.bfloat16, isOutput=False)
    # RoPE half tables: rows d<64 of cos/sin transposed (rows 64-127 are
    # identical by the rotate-half construction; duplicated on-chip)
    cos_h = nc.declare_dram_parameter("cos_h", [DH // 2, S], dt.bfloat16, isOutput=False)
    sin_h = nc.declare_dram_parameter("sin_h", [DH // 2, S], dt.bfloat16, isOutput=False)
    # tri[p, c] = 1.0 if p <= c else 0.0  (128x128 causal triangle)
    tri = nc.declare_dram_parameter("tri", [128, 128], dt.bfloat16, isOutput=False)
    ident = nc.declare_dram_parameter("ident", [128, 128], dt.bfloat16, isOutput=False)
    out = nc.declare_dram_parameter("out", [S, DIN], dt.bfloat16, isOutput=True)

    with tile.TileContext(nc) as tc, ExitStack() as ctx:
        singles = ctx.enter_context(tc.tile_pool(name="singles", bufs=1))
        wop = ctx.enter_context(tc.tile_pool(name="wop", bufs=1))
        xp = ctx.enter_context(tc.tile_pool(name="xp", bufs=2))
        qkv = ctx.enter_context(tc.tile_pool(name="qkv", bufs=1))
        epool = ctx.enter_context(tc.tile_pool(name="epool", bufs=5))
        spool = ctx.enter_context(tc.tile_pool(name="spool", bufs=3))
        npool = ctx.enter_context(tc.tile_pool(name="npool", bufs=2))
        tpool = ctx.enter_context(tc.tile_pool(name="tpool", bufs=2))
        obp = ctx.enter_context(tc.tile_pool(name="obp", bufs=3))
        # One PSUM pool, four 2-slot tag arenas = all 8 banks.
        psum = ctx.enter_context(tc.tile_pool(name="psum", bufs=2, space="PSUM"))

        def ps_tile(tag, shape=(128, QC), dtyp=dt.float32, name="ps"):
            return psum.tile(list(shape), dtyp, tag=tag, name=name)

        # ---- SBUF residents ----
        wall_t = singles.tile([128, NK, NS, DH], dt.bfloat16, tag="wall")
        c_cos = singles.tile([DH, S], dt.bfloat16, tag="cos")
        c_sin = singles.tile([DH, S], dt.bfloat16, tag="sin")
        c_tri = singles.tile([128, 128], dt.bfloat16, tag="tri")
        c_id = singles.tile([128, 128], dt.bfloat16, tag="ident")
        w_o = wop.tile([128, HPC, DIN], dt.bfloat16, tag="wo")

        xcs0 = [xp.tile([128, CW, QC], dt.bfloat16, tag=f"xc{g}",
                        name=f"xc{g}_0") for g in range(NXC)]

        # ---- early DMA schedule on two queues, in consumption order ----
        # Quarter-0 runs chunk-synchronous waves (all six streams consume
        # chunk w together, ~5us per chunk), so deliveries need only keep
        # that pace. sync (SP) queue: the x chunks + wall group 1;
        # scalar (Activation) queue: the remaining wall groups, cos/sin, wo.
        def wall_grp(g, s0_, s1_, eng):
            eng.dma_start(out=wall_t[:, g * CW:(g + 1) * CW, s0_:s1_],
                          in_=wall[:, g * CW:(g + 1) * CW, s0_:s1_])

        # every wall group is split (a = K,V; b = Q0..Q3): the wave's K/V
        # matmuls gate on the small a-piece instead of the whole 768KB group
        wall_grp(0, 0, 2, nc.scalar)
        nc.sync.dma_start(out=xcs0[0], in_=x[:, 0, 0:CW])
        wall_grp(0, 2, NS, nc.scalar)
        nc.sync.dma_start(out=xcs0[1], in_=x[:, 0, CW:2 * CW])
        wall_grp(2, 0, 2, nc.scalar)
        wall_grp(1, 0, 2, nc.sync)
        wall_grp(1, 2, NS, nc.sync)
        wall_grp(2, 2, NS, nc.scalar)
        wall_grp(3, 0, 2, nc.scalar)
        wall_grp(3, 2, NS, nc.scalar)
        for g in range(2, NXC):
            nc.sync.dma_start(out=xcs0[g], in_=x[:, 0, g * CW:(g + 1) * CW])
            if g == 3:
                wall_grp(4, 0, 2, nc.scalar)
                wall_grp(4, 2, NS, nc.scalar)
            elif g == 4:
                nc.scalar.dma_start(out=c_cos[0:64], in_=cos_h[:])
                nc.scalar.dma_start(out=c_sin[64:128], in_=sin_h[:])
            elif g >= 5:
                wall_grp(g, 0, 2, nc.scalar)
                wall_grp(g, 2, NS, nc.scalar)
        nc.sync.dma_start(out=c_tri, in_=tri[:])
        nc.sync.dma_start(out=c_id, in_=ident[:])
        for h in range(HPC):
            nc.scalar.dma_start(out=w_o[:, h], in_=wo[:, h])

        # ---- PE warm-up + exp-bias constant ----
        # A serial chain of tiny matmuls during the initial DMA wait keeps
        # the PE busy (the dependency chain paces it at latency rate, which
        # is the point: coverage, not throughput) so the HAM clock-gate
        # releases before the first real matmul. The chain accumulates zeros
        # and finally ones^T @ (EXP_BIAS/128), producing the exp bias
        # vector -- a live chain, so nothing is DCE'd.
        w1 = singles.tile([128, 128], dt.bfloat16, tag="warm1")
        nc.vector.memset(w1, 1.0)
        wz = singles.tile([128, 1], dt.bfloat16, tag="warmz")
        nc.vector.memset(wz, 0.0)
        wb = singles.tile([128, 1], dt.bfloat16, tag="warmb")
        nc.vector.memset(wb, EXP_BIAS / 128.0)
        ps_warm = ps_tile("sc", (128, 1), name="ps_warm")
        for i in range(NWARM):
            nc.tensor.matmul(ps_warm, lhsT=w1,
                             rhs=(wz if i < NWARM - 1 else wb),
                             start=(i == 0), stop=(i == NWARM - 1))
        c_bias = singles.tile([128, 1], dt.float32, tag="ebias")
        nc.scalar.copy(c_bias, ps_warm)

        # derive full RoPE tables (upper cos half copy; lower sin half negate)
        nc.vector.tensor_copy(c_cos[64:128], c_cos[0:64])
        nc.vector.tensor_scalar_mul(c_sin[0:64], c_sin[64:128], -1.0)

        # ---- long-lived activations ----
        # qt: one [DH, QC] buffer per head, rewritten each quarter.
        # ctxT: two quarters per head (blocks of quarter sq are consumed
        # during quarter sq+1 while sq+1's attention writes the other half).
        kt = qkv.tile([DH, S], dt.bfloat16, tag="kt")
        vn = qkv.tile([128, NKT, DH], dt.bfloat16, tag="vn")   # V natural tiles
        ctxT = [qkv.tile([DH, 2, QC], dt.bfloat16, tag=f"ctx{h}", name=f"ctx{h}")
                for h in range(HPC)]
        cur_qt = [None] * HPC

        def new_qt(h):
            t = qkv.tile([DH, QC], dt.bfloat16, tag=f"qt{h}", name=f"qt{h}")
            cur_qt[h] = t
            return t

        def rope_from_psum(ps, dst_slice, s0):
            """dst = ps*cos + rot_half(ps)*sinm over s-columns [s0, s0+QC)."""
            t1 = tpool.tile([DH, QC], dt.float32, tag="t1", name="t1")
            nc.vector.tensor_mul(t1, ps, c_cos[:, s0:s0 + QC])
            t2 = tpool.tile([DH, QC], dt.float32, tag="t2", name="t2")
            nc.vector.tensor_mul(t2[0:64, :], ps[64:128, :], c_sin[0:64, s0:s0 + QC])
            nc.vector.tensor_mul(t2[64:128, :], ps[0:64, :], c_sin[64:128, s0:s0 + QC])
            nc.vector.tensor_add(dst_slice, t1, t2)

        def v_drain(psv):
            """V PSUM -> bf16 (scalar engine), releasing the PSUM slot."""
            vtmp = tpool.tile([DH, QC], dt.bfloat16, tag="vtmp", name="vtmp")
            nc.scalar.copy(vtmp, psv)
            return vtmp

        def v_transposes(sq, vtmp):
            """128x128 transposes into vn. The pvt->vn copies alternate
            between the scalar engine and the DVE so neither the upcoming
            exp stream (scalar) nor the RoPE chain (DVE) is pushed back by
            the full set."""
            for i in range(QC // 128):
                pvt = ps_tile("sc", (128, 128), dt.bfloat16, name="pvt")
                nc.tensor.transpose(pvt, vtmp[:, i * 128:(i + 1) * 128], c_id)
                if i % 2 == 0:
                    nc.scalar.copy(vn[:, sq * 4 + i], pvt)
                else:
                    nc.vector.tensor_copy(vn[:, sq * 4 + i], pvt)

        kv_box = [None]   # psv of the pumped next-quarter V projection

        def kv_steps(sq2, xref):
            """Generator: the NEXT quarter's K and V projections, pumped as
            PE filler into the current quarter's attention. RoPE(K) is
            emitted inline; the V transposes are left to the caller (their
            pvt slots and vn copies would collide with the exp stream).
            xref is a 1-element box so prefetched chunks can be supplied
            after the generator is constructed."""
            s02 = sq2 * QC
            psk = ps_tile("proj", name="psk")
            for k in range(NK):
                nc.tensor.matmul(psk, lhsT=wall_t[:, k, 0],
                                 rhs=xref[0][k // CW][:, k % CW],
                                 start=(k == 0), stop=(k == NK - 1))
                yield
            rope_from_psum(psk, kt[:, s02:s02 + QC], s02)
            psv = ps_tile("proj", name="psv")
            for k in range(NK):
                nc.tensor.matmul(psv, lhsT=wall_t[:, k, 1],
                                 rhs=xref[0][k // CW][:, k % CW],
                                 start=(k == 0), stop=(k == NK - 1))
                yield
            kv_box[0] = psv

        def emit_q(sq, h, xcs, acc_tag="po", fill=None):
            """Q projection + RoPE for head h, quarter sq. A couple of filler
            matmuls are pumped BEFORE the rope is emitted so the fillers'
            PSUM-WAR drains run ahead of the rope in the DVE queue; the rest
            cover the rope's latency before the first score matmul."""
            s0 = sq * QC
            psq = ps_tile(acc_tag, name="psq")
            for k in range(NK):
                nc.tensor.matmul(psq, lhsT=wall_t[:, k, 2 + h],
                                 rhs=xcs[k // CW][:, k % CW],
                                 start=(k == 0), stop=(k == NK - 1))
            if fill is not None:
                pump(fill, 2)
            rope_from_psum(psq, new_qt(h), s0)
            if fill is not None:
                pump(fill, 6)

        def emit_attn_head(sq, h, po_tag="po", filler=None):
            """Causal attention for head h over quarter sq's queries.

            Scores are [key-tile, q] transposed; diagonal key-tiles are
            trimmed to the columns that aren't fully masked, and the
            128-wide triangle on the diagonal gets the 0/1 mask."""
            s0 = sq * QC
            njt = 4 * (sq + 1)
            sacc = spool.tile([128, QC], dt.bfloat16, tag="sacc", name="sacc")
            po = ps_tile(po_tag, name="po")
            for jt in range(njt):
                r = jt - (njt - 4)          # >=0 -> diagonal tile index
                c0 = 128 * r if r > 0 else 0
                psc = ps_tile("sc", name="psc")
                nc.tensor.matmul(psc[:, c0:QC], lhsT=kt[:, jt * KT:(jt + 1) * KT],
                                 rhs=cur_qt[h][:, c0:QC],
                                 start=True, stop=True)
                e = epool.tile([128, QC], dt.bfloat16, tag="e", name="e")
                nc.scalar.activation(out=e[:, c0:QC], in_=psc[:, c0:QC],
                                     func=mybir.ActivationFunctionType.Exp,
                                     bias=c_bias, scale=SCALE)
                if r >= 0:
                    nc.vector.tensor_mul(e[:, c0:c0 + 128], e[:, c0:c0 + 128],
                                         c_tri)
                if jt == 0:
                    nc.vector.tensor_copy(sacc, e[:, 0:QC])
                else:
                    nc.vector.tensor_add(sacc[:, c0:QC], sacc[:, c0:QC],
                                         e[:, c0:QC])
                if filler is not None:
                    # one filler matmul per attention tile, placed BETWEEN
                    # the score and PV matmuls so it runs inside the exp
                    # latency window (exp is ~1.5x slower per tile than the
                    # sc+pv pair)
                    pump(filler, 1)
                nc.tensor.matmul(po[:, c0:QC], lhsT=vn[:, jt], rhs=e[:, c0:QC],
                                 start=(jt == 0), stop=(jt == njt - 1))
            # normalization: ctx = O * (1/colsum(E)). colsum broadcast to all
            # 128 partitions by a single ones[128,128] matmul (the all-ones
            # warmup tile), then a fast approximate reciprocal and one DVE
            # scale. Short chain, nothing on gpsimd.
            pcsb = ps_tile("sc", name="pcsb")
            nc.tensor.matmul(pcsb, lhsT=w1, rhs=sacc, start=True, stop=True)
            rec = npool.tile([128, QC], dt.float32, tag="rec", name="rec")
            nc.vector.reciprocal_approx_fast(out=rec, in_=pcsb)
            nc.vector.tensor_mul(ctxT[h][:, sq % 2, :], po, rec)

        def outproj_steps(st, pool_tag="out", dma_q="sync", dve_drains=False):
            """Generator form of the out-projection block
            out[st*128:(st+1)*128, :] = sum_h ctxT[h][:, st-block].T @ Wo[h];
            yields after each matmul so it can be pumped as PE filler inside
            the attention loop. With dve_drains, PSUM drains stay off the
            scalar engine (busy with exp) entirely."""
            half, colq = (st // 4) % 2, (st % 4) * 128
            for quad in range(4):
                ob = obp.tile([128, DIN // 4], dt.bfloat16, tag="ob", name="ob")
                for j in range(2):
                    oc = quad * 2 + j
                    tag = (pool_tag[oc % 2]
                           if isinstance(pool_tag, tuple) else pool_tag)
                    pso = ps_tile(tag, (128, 512), name="pso")
                    for h in range(HPC):
                        nc.tensor.matmul(pso,
                                         lhsT=ctxT[h][:, half, colq:colq + 128],
                                         rhs=w_o[:, h, oc * 512:(oc + 1) * 512],
                                         start=(h == 0), stop=(h == HPC - 1))
                        yield
                    # drain right away: the PSUM slot must be free well
                    # before the pumped rotation reaches it again
                    dst = ob[:, j * 512:(j + 1) * 512]
                    if dve_drains or oc % 2:
                        nc.vector.tensor_copy(dst, pso)
                    else:
                        nc.scalar.copy(dst, pso)
                eng = nc.sync if (dma_q == "sync" or quad % 2 == 0) else nc.scalar
                eng.dma_start(
                    out=out[st * 128:(st + 1) * 128,
                            quad * (DIN // 4):(quad + 1) * (DIN // 4)],
                    in_=ob)

        def emit_outproj_block(st, pool_tag="out", dma_q="sync",
                               dve_drains=False):
            for _ in outproj_steps(st, pool_tag, dma_q, dve_drains):
                pass

        _DONE = object()

        def pump(it, n):
            for _ in range(n):
                if next(it, _DONE) is _DONE:
                    return

        def prefetch_x(sq1, gs, queue="sync"):
            """Load x chunks for quarter sq1 (double-buffered: no WAR wait)."""
            eng = nc.sync if queue == "sync" else nc.scalar
            tiles = []
            for g in gs:
                xc = xp.tile([128, CW, QC], dt.bfloat16,
                             tag=f"xc{g}", name=f"xc{g}_{sq1}")
                eng.dma_start(out=xc, in_=x[:, sq1, g * CW:(g + 1) * CW])
                tiles.append(xc)
            return tiles

        # ---- quarter 0: chunk-synchronous projection waves ----
        # All six streams consume chunk w together (24 matmuls ~ 5us per
        # chunk), so the cold-start DMA keeps pace and the PE stays hot
        # from the first chunk on. In the last wave each stream's epilogue
        # (RoPE / V handoff) is emitted right after its final matmul so the
        # DVE works through the epilogues while the PE finishes the wave.
        accs = [ps_tile("proj", name="wavK"), ps_tile("proj", name="wavV"),
                ps_tile("po", name="wavQ0"), ps_tile("po", name="wavQ1"),
                ps_tile("out", name="wavQ2"), ps_tile("out", name="wavQ3")]
        vtmp0 = None
        for w in range(NXC):
            for s in range(NS):
                for kk in range(CW):
                    k = w * CW + kk
                    nc.tensor.matmul(accs[s], lhsT=wall_t[:, k, s],
                                     rhs=xcs0[w][:, kk],
                                     start=(k == 0), stop=(k == NK - 1))
                if w == NXC - 1:
                    if s == 0:
                        rope_from_psum(accs[0], kt[:, 0:QC], 0)
                    elif s == 1:
                        vtmp0 = tpool.tile([DH, QC], dt.bfloat16, tag="vtmp",
                                           name="vtmp")
                        nc.scalar.copy(vtmp0, accs[1])
                    elif s == 2:
                        rope_from_psum(accs[2], new_qt(0), 0)
            if w == 4:
                nxt = prefetch_x(1, range(0, 4), queue="sync")
            elif w == 6:
                nxt += prefetch_x(1, range(4, NXC), queue="sync")
        for i in range(QC // 128):
            pvt = ps_tile("sc", (128, 128), dt.bfloat16, name="pvt")
            nc.tensor.transpose(pvt, vtmp0[:, i * 128:(i + 1) * 128], c_id)
            if i % 2 == 0:
                nc.scalar.copy(vn[:, i], pvt)
            else:
                nc.vector.tensor_copy(vn[:, i], pvt)
        # quarter-1 K/V projections pumped as PE filler into quarter 0's
        # attention -- each head's attention alone (~1.1us) cannot cover its
        # RoPE latency (~2.1us of serial DVE), so the heads would otherwise
        # drift apart waiting on the DVE.
        fill0 = kv_steps(1, [nxt])
        rope_from_psum(accs[3], new_qt(1), 0)                  # Q1
        emit_attn_head(0, 0, po_tag="proj")
        rope_from_psum(accs[4], new_qt(2), 0)                  # Q2
        emit_attn_head(0, 1, po_tag="proj", filler=fill0)
        pump(fill0, 10)
        rope_from_psum(accs[5], new_qt(3), 0)                  # Q3
        emit_attn_head(0, 2, po_tag="po", filler=fill0)
        pump(fill0, 10)
        emit_attn_head(0, 3, po_tag="po", filler=fill0)
        for _ in fill0:
            pass
        vtmp1 = v_drain(kv_box[0])
        emit_outproj_block(0, pool_tag="out")
        v_transposes(1, vtmp1)

        # ---- quarters 1..3: stream-serial (x already resident) ----
        # The previous quarter's three inner out-proj blocks plus the NEXT
        # quarter's K/V projections are pumped as fine-grained PE filler:
        # ~8 matmuls cover each head's Q->RoPE latency, one matmul per
        # attention tile absorbs the exp lag, and the chained K/V work
        # keeps the last head's attention fed.
        xcs = nxt
        for sq in range(1, NQ):
            nxt_ref = [None]
            gens = [outproj_steps(4 * (sq - 1) + 1, "out", dve_drains=True),
                    outproj_steps(4 * (sq - 1) + 2, "proj", dve_drains=True),
                    outproj_steps(4 * (sq - 1) + 3, "out", dve_drains=True)]
            if sq + 1 < NQ:
                gens.append(kv_steps(sq + 1, nxt_ref))
            fill = itertools.chain(*gens)
            for h in range(HPC):
                emit_q(sq, h, xcs, fill=fill)
                if h == 1 and sq + 1 < NQ:
                    nxt_ref[0] = prefetch_x(sq + 1, range(0, 4), queue="sync")
                if h == 2 and sq + 1 < NQ:
                    nxt_ref[0] = nxt_ref[0] + prefetch_x(sq + 1, range(4, NXC),
                                                         queue="sync")
                emit_attn_head(sq, h, filler=fill)
                pump(fill, 8)
            for _ in fill:
                pass
            if sq + 1 < NQ:
                vtmp_n = v_drain(kv_box[0])
            # quarter-boundary block on the "sc"/"proj" arenas (free after
            # the last head's attention); DVE drains -- the scalar engine
            # still has the last head's exp backlog
            emit_outproj_block(4 * sq, pool_tag=("sc", "proj"),
                               dma_q=("alt" if sq == NQ - 1 else "sync"),
                               dve_drains=True)
            if sq + 1 < NQ:
                v_transposes(sq + 1, vtmp_n)
                xcs = nxt_ref[0]
        # ---- tail blocks: 4-deep psum rotation, alternate output queues ----
        for st, tag in ((13, ("out", "po")), (14, ("proj", "sc")),
                        (15, ("out", "po"))):
            emit_outproj_block(st, pool_tag=tag, dma_q="alt")
    nc.finalize()
    return nc


def make_in_maps(input_tensor, cos, sin, Wq, Wk, Wv, Wo):
    """Host-side sharding + layout preparation. Returns list of 8 dicts."""
    x2 = np.ascontiguousarray(input_tensor.reshape(S, DIN))
    # x_host[p, sq, k, sc] = x2[512*sq+sc, 128*k+p]
    xt = x2.T.astype(BF16)                      # [DIN, S]
    x_host = np.ascontiguousarray(
        xt.reshape(NK, 128, NQ, QC).transpose(1, 2, 0, 3))

    cos_h = np.ascontiguousarray(cos.T[0:64].astype(BF16))
    sin_h = np.ascontiguousarray(sin.T[0:64].astype(BF16))

    p_idx = np.arange(128)[:, None]
    c_idx = np.arange(128)[None, :]
    tri = (p_idx <= c_idx).astype(BF16)
    ident = np.eye(128, dtype=BF16)

    common = dict(x=x_host, cos_h=cos_h, sin_h=sin_h, tri=tri, ident=ident)

    in_maps = []
    for c in range(NCORES):
        wq_s = Wq[:, c * DPC:(c + 1) * DPC].astype(BF16)
        wq_host = wq_s.reshape(NK, 128, HPC, DH).transpose(1, 0, 2, 3)
        wk_s = Wk[:, c * DH:(c + 1) * DH].astype(BF16)
        wk_host = wk_s.reshape(NK, 128, DH).transpose(1, 0, 2)
        wv_s = Wv[:, c * DH:(c + 1) * DH].astype(BF16)
        wv_host = wv_s.reshape(NK, 128, DH).transpose(1, 0, 2)
        # wall[p, k, s, d]: s = (K, V, Q0..Q3)
        wall_host = np.ascontiguousarray(np.stack(
            [wk_host, wv_host] + [wq_host[:, :, h] for h in range(HPC)], axis=2))
        wo_s = Wo[c * DPC:(c + 1) * DPC, :].astype(BF16)
        wo_host = np.ascontiguousarray(wo_s.reshape(HPC, 128, DIN).transpose(1, 0, 2))
        in_maps.append(dict(common, wall=wall_host, wo=wo_host))
    return in_maps


def _numpy_fallback(input_tensor, attention_mask, cos, sin, Wq, Wk, Wv, Wo):
    x = input_tensor.astype(np.float32)
    b, s, _ = x.shape
    q = (x @ Wq).reshape(b, s, H, DH).transpose(0, 2, 1, 3)
    k = (x @ Wk).reshape(b, s, KV, DH).transpose(0, 2, 1, 3)
    v = (x @ Wv).reshape(b, s, KV, DH).transpose(0, 2, 1, 3)

    def rope(t):
        t1, t2 = t[..., :64], t[..., 64:]
        rot = np.concatenate([-t2, t1], axis=-1)
        return t * cos[None, None] + rot * sin[None, None]

    q, k = rope(q), rope(k)
    k = np.repeat(k, G, axis=1)
    v = np.repeat(v, G, axis=1)
    sc = np.einsum('bhqd,bhkd->bhqk', q, k)
    sc = np.where(attention_mask, -np.inf, sc) / np.float32(np.sqrt(DH))
    sc = sc - sc.max(axis=-1, keepdims=True)
    w = np.exp(sc)
    w = w / w.sum(axis=-1, keepdims=True)
    ctx = np.einsum('bhqk,bhkd->bhqd', w, v)
    ctx = ctx.transpose(0, 2, 1, 3).reshape(b, s, H * DH)
    return (ctx @ Wo).astype(np.float32)


_NC_CACHE = {}


def kernel(input_tensor, attention_mask, cos, sin, Wq, Wk, Wv, Wo):
    mask = np.asarray(attention_mask).reshape(S, S)
    causal = np.array_equal(mask, np.triu(np.ones((S, S), bool), k=1))
    cos_a, sin_a = np.asarray(cos), np.asarray(sin)
    halved = (np.array_equal(cos_a[:, :64], cos_a[:, 64:])
              and np.array_equal(sin_a[:, :64], sin_a[:, 64:]))
    if not (causal and halved):
        return _numpy_fallback(np.asarray(input_tensor), np.asarray(attention_mask),
                               np.asarray(cos), np.asarray(sin),
                               np.asarray(Wq), np.asarray(Wk),
                               np.asarray(Wv), np.asarray(Wo))

    if "nc" not in _NC_CACHE:
        _NC_CACHE["nc"] = build_nc()
    nc = _NC_CACHE["nc"]

    in_maps = make_in_maps(np.asarray(input_tensor), np.asarray(cos),
                           np.asarray(sin), np.asarray(Wq), np.asarray(Wk),
                           np.asarray(Wv), np.asarray(Wo))
    res = run_bass_kernel_spmd(nc, in_maps, core_ids=list(range(NCORES)))
    acc = np.zeros((S, DIN), np.float32)
    for r in res.results:
        acc += np.asarray(r["out"], dtype=np.float32)
    return acc.reshape(1, S, DIN)


# revision 42
# speedup vs baseline: 1.1585x; 1.1585x over previous
"""GroupQueryAttention on 8 TRN2 NeuronCores.

Strategy: tensor-parallel over heads. H=32 query heads, KV=8 kv heads,
group size G=4 -> each core owns exactly 1 kv head and its 4 query heads.
Per core:
  - QKV projections from a replicated (pre-transposed, channels-major) input
  - RoPE on Q/K (rotate-half, done on DVE across partition halves)
  - attention with scores computed TRANSPOSED ([keys, q] layout) so the
    exp(scores) tiles feed the V-matmul directly as the moving operand;
    softmax normalization is deferred: O = V.E, then ctx = O * (1/colsum(E))
  - partial output ctx @ Wo_shard  (row-shard of Wo)
Host sums the 8 partial outputs (the "all-reduce" of the row-parallel Wo).

Perf structure (v2):
  - dual DMA queues: loads are split between the SP (sync) and Activation
    (scalar) hardware DGE queues -- the single-queue serialization of v1
    starved the PE at kernel start and backed up the output at the end
  - quarter-0 projections are emitted as a chunk-major WAVE across the six
    projection streams (K, V, Q0..Q3, weights interleaved per k-group in
    one "wall" tensor) so the PE consumes each arriving x chunk at ~1/6th
    the single-stream rate -- the cold-start DMA can keep up and the PE
    never idles (idle triggers a ~7us half-speed HAM window)
  - x chunks double-buffered (bufs=2) so next-quarter prefetch has no WAR
    dependency and can be issued early on either queue
  - softmax normalization: colsum is computed pre-broadcast by a single
    ones[128,128] matmul (same cost as the old ones-column matmul), then
    DVE reciprocal_approx_fast + DVE scale. No gpsimd hop (the
    partition_broadcast custom op had ~1.2us latency on the critical path)
  - out-projection PSUM allocations rotate across three pool tags so a
    block's matmuls never wait on drains queued behind the previous head's
    exp avalanche; tail blocks' output DMAs alternate queues
  - a warm-up chain of tiny matmuls (computing the exp bias constant)
    lifts the PE HAM clock gate during the initial DMA wait
"""

import itertools
import sys

sys.path.insert(0, "/opt/trn_rl_repo")

from contextlib import ExitStack

import numpy as np
import ml_dtypes

import concourse.bass as bass
import concourse.bacc as bacc
import concourse.tile as tile
from concourse import mybir
from concourse.bass_utils import run_bass_kernel_spmd

BF16 = ml_dtypes.bfloat16

S = 2048          # sequence length
DIN = 4096        # model dim
H, KV, DH = 32, 8, 128
G = H // KV       # 4 query heads per kv head
NCORES = 8
HPC = H // NCORES     # 4 query heads per core
DPC = HPC * DH        # 512 = per-core q-projection width

NQ = 4            # s-quarters (chunks of 512 queries)
QC = S // NQ      # 512
KT = 128          # key tile (partition dim of transposed scores)
NKT = S // KT     # 16 key tiles
NK = DIN // 128   # 32 contraction tiles for projections
NXC = 8           # x chunks per quarter (k-groups of CW)
CW = NK // NXC    # 4 k-tiles per x chunk
NS = 6            # projection streams: K, V, Q0..Q3
SCALE = 1.0 / float(np.sqrt(DH))
EXP_BIAS = -10.0  # constant shift inside exp; cancels in normalization
NWARM = 80


def build_nc():
    """Build the per-core Bass program (same program on all 8 cores; the
    per-core weight shards arrive via in_maps)."""
    nc = bacc.Bacc()
    dt = mybir.dt

    # ---- DRAM parameters (host-prepared layouts; all DMA-contiguous) ----
    # x[p, sq, k, sc] = x_orig[512*sq + sc, 128*k + p]   (channels-major)
    x = nc.declare_dram_parameter("x", [128, NQ, NK, QC], dt.bfloat16, isOutput=False)
    # wall[p, k, s, d]: s=0 -> Wk_shard, s=1 -> Wv_shard, s=2+h -> Wq head h
    wall = nc.declare_dram_parameter("wall", [128, NK, NS, DH], dt.bfloat16,
                                     isOutput=False)
    # wo[p, h, n] = Wo_shard[128*h + p, n]
    wo = nc.declare_dram_parameter("wo", [128, HPC, DIN], dt.bfloat16, isOutput=False)
    # RoPE half tables: rows d<64 of cos/sin transposed (rows 64-127 are
    # identical by the rotate-half construction; duplicated on-chip)
    cos_h = nc.declare_dram_parameter("cos_h", [DH // 2, S], dt.bfloat16, isOutput=False)
    sin_h = nc.declare_dram_parameter("sin_h", [DH // 2, S], dt.bfloat16, isOutput=False)
    # tri[p, c] = 1.0 if p <= c else 0.0  (128x128 causal triangle)
    tri = nc.declare_dram_parameter("tri", [128, 128], dt.bfloat16, isOutput=False)
    ident = nc.declare_dram_parameter("ident", [128, 128], dt.bfloat16, isOutput=False)
    out = nc.declare_dram_parameter("out", [S, DIN], dt.bfloat16, isOutput=True)

    with tile.TileContext(nc) as tc, ExitStack() as ctx:
        singles = ctx.enter_context(tc.tile_pool(name="singles", bufs=1))
        wop = ctx.enter_context(tc.tile_pool(name="wop", bufs=1))
        xp = ctx.enter_context(tc.tile_pool(name="xp", bufs=2))
        qkv = ctx.enter_context(tc.tile_pool(name="qkv", bufs=1))
        epool = ctx.enter_context(tc.tile_pool(name="epool", bufs=5))
        spool = ctx.enter_context(tc.tile_pool(name="spool", bufs=3))
        npool = ctx.enter_context(tc.tile_pool(name="npool", bufs=2))
        tpool = ctx.enter_context(tc.tile_pool(name="tpool", bufs=2))
        obp = ctx.enter_context(tc.tile_pool(name="obp", bufs=3))
        # One PSUM pool, four 2-slot tag arenas = all 8 banks.
        psum = ctx.enter_context(tc.tile_pool(name="psum", bufs=2, space="PSUM"))

        def ps_tile(tag, shape=(128, QC), dtyp=dt.float32, name="ps"):
            return psum.tile(list(shape), dtyp, tag=tag, name=name)

        # ---- SBUF residents ----
        wall_t = singles.tile([128, NK, NS, DH], dt.bfloat16, tag="wall")
        c_cos = singles.tile([DH, S], dt.bfloat16, tag="cos")
        c_sin = singles.tile([DH, S], dt.bfloat16, tag="sin")
        c_tri = singles.tile([128, 128], dt.bfloat16, tag="tri")
        c_id = singles.tile([128, 128], dt.bfloat16, tag="ident")
        w_o = wop.tile([128, HPC, DIN], dt.bfloat16, tag="wo")

        xcs0 = [xp.tile([128, CW, QC], dt.bfloat16, tag=f"xc{g}",
                        name=f"xc{g}_0") for g in range(NXC)]

        # ---- early DMA schedule on two queues, in consumption order ----
        # Quarter-0 runs chunk-synchronous waves (all six streams consume
        # chunk w together, ~5us per chunk), so deliveries need only keep
        # that pace. sync (SP) queue: the x chunks + wall group 1;
        # scalar (Activation) queue: the remaining wall groups, cos/sin, wo.
        def wall_grp(g, s0_, s1_, eng):
            eng.dma_start(out=wall_t[:, g * CW:(g + 1) * CW, s0_:s1_],
                          in_=wall[:, g * CW:(g + 1) * CW, s0_:s1_])

        wall_grp(0, 0, 2, nc.scalar)
        nc.sync.dma_start(out=xcs0[0], in_=x[:, 0, 0:CW])
        wall_grp(0, 2, NS, nc.scalar)
        nc.sync.dma_start(out=xcs0[1], in_=x[:, 0, CW:2 * CW])
        wall_grp(2, 0, NS, nc.scalar)
        wall_grp(1, 0, NS, nc.sync)
        wall_grp(3, 0, NS, nc.scalar)
        for g in range(2, NXC):
            nc.sync.dma_start(out=xcs0[g], in_=x[:, 0, g * CW:(g + 1) * CW])
            if g == 3:
                wall_grp(4, 0, NS, nc.scalar)
            elif g == 4:
                nc.scalar.dma_start(out=c_cos[0:64], in_=cos_h[:])
                nc.scalar.dma_start(out=c_sin[64:128], in_=sin_h[:])
            elif g >= 5:
                wall_grp(g, 0, NS, nc.scalar)
        nc.sync.dma_start(out=c_tri, in_=tri[:])
        nc.sync.dma_start(out=c_id, in_=ident[:])
        for h in range(HPC):
            nc.scalar.dma_start(out=w_o[:, h], in_=wo[:, h])

        # ---- PE warm-up + exp-bias constant ----
        # A serial chain of tiny matmuls during the initial DMA wait keeps
        # the PE busy (the dependency chain paces it at latency rate, which
        # is the point: coverage, not throughput) so the HAM clock-gate
        # releases before the first real matmul. The chain accumulates zeros
        # and finally ones^T @ (EXP_BIAS/128), producing the exp bias
        # vector -- a live chain, so nothing is DCE'd.
        w1 = singles.tile([128, 128], dt.bfloat16, tag="warm1")
        nc.vector.memset(w1, 1.0)
        wz = singles.tile([128, 1], dt.bfloat16, tag="warmz")
        nc.vector.memset(wz, 0.0)
        wb = singles.tile([128, 1], dt.bfloat16, tag="warmb")
        nc.vector.memset(wb, EXP_BIAS / 128.0)
        ps_warm = ps_tile("sc", (128, 1), name="ps_warm")
        for i in range(NWARM):
            nc.tensor.matmul(ps_warm, lhsT=w1,
                             rhs=(wz if i < NWARM - 1 else wb),
                             start=(i == 0), stop=(i == NWARM - 1))
        c_bias = singles.tile([128, 1], dt.float32, tag="ebias")
        nc.scalar.copy(c_bias, ps_warm)

        # derive full RoPE tables (upper cos half copy; lower sin half negate)
        nc.vector.tensor_copy(c_cos[64:128], c_cos[0:64])
        nc.vector.tensor_scalar_mul(c_sin[0:64], c_sin[64:128], -1.0)

        # ---- long-lived activations ----
        # qt: one [DH, QC] buffer per head, rewritten each quarter.
        # ctxT: two quarters per head (blocks of quarter sq are consumed
        # during quarter sq+1 while sq+1's attention writes the other half).
        kt = qkv.tile([DH, S], dt.bfloat16, tag="kt")
        vn = qkv.tile([128, NKT, DH], dt.bfloat16, tag="vn")   # V natural tiles
        ctxT = [qkv.tile([DH, 2, QC], dt.bfloat16, tag=f"ctx{h}", name=f"ctx{h}")
                for h in range(HPC)]
        cur_qt = [None] * HPC

        def new_qt(h):
            t = qkv.tile([DH, QC], dt.bfloat16, tag=f"qt{h}", name=f"qt{h}")
            cur_qt[h] = t
            return t

        def rope_from_psum(ps, dst_slice, s0):
            """dst = ps*cos + rot_half(ps)*sinm over s-columns [s0, s0+QC)."""
            t1 = tpool.tile([DH, QC], dt.float32, tag="t1", name="t1")
            nc.vector.tensor_mul(t1, ps, c_cos[:, s0:s0 + QC])
            t2 = tpool.tile([DH, QC], dt.float32, tag="t2", name="t2")
            nc.vector.tensor_mul(t2[0:64, :], ps[64:128, :], c_sin[0:64, s0:s0 + QC])
            nc.vector.tensor_mul(t2[64:128, :], ps[0:64, :], c_sin[64:128, s0:s0 + QC])
            nc.vector.tensor_add(dst_slice, t1, t2)

        def v_drain(psv):
            """V PSUM -> bf16 (scalar engine), releasing the PSUM slot."""
            vtmp = tpool.tile([DH, QC], dt.bfloat16, tag="vtmp", name="vtmp")
            nc.scalar.copy(vtmp, psv)
            return vtmp

        def v_transposes(sq, vtmp):
            """128x128 transposes into vn. The pvt->vn copies alternate
            between the scalar engine and the DVE so neither the upcoming
            exp stream (scalar) nor the RoPE chain (DVE) is pushed back by
            the full set."""
            for i in range(QC // 128):
                pvt = ps_tile("sc", (128, 128), dt.bfloat16, name="pvt")
                nc.tensor.transpose(pvt, vtmp[:, i * 128:(i + 1) * 128], c_id)
                if i % 2 == 0:
                    nc.scalar.copy(vn[:, sq * 4 + i], pvt)
                else:
                    nc.vector.tensor_copy(vn[:, sq * 4 + i], pvt)

        kv_box = [None]   # psv of the pumped next-quarter V projection

        def kv_steps(sq2, xref):
            """Generator: the NEXT quarter's K and V projections, pumped as
            PE filler into the current quarter's attention. RoPE(K) is
            emitted inline; the V transposes are left to the caller (their
            pvt slots and vn copies would collide with the exp stream).
            xref is a 1-element box so prefetched chunks can be supplied
            after the generator is constructed."""
            s02 = sq2 * QC
            psk = ps_tile("proj", name="psk")
            for k in range(NK):
                nc.tensor.matmul(psk, lhsT=wall_t[:, k, 0],
                                 rhs=xref[0][k // CW][:, k % CW],
                                 start=(k == 0), stop=(k == NK - 1))
                yield
            rope_from_psum(psk, kt[:, s02:s02 + QC], s02)
            psv = ps_tile("proj", name="psv")
            for k in range(NK):
                nc.tensor.matmul(psv, lhsT=wall_t[:, k, 1],
                                 rhs=xref[0][k // CW][:, k % CW],
                                 start=(k == 0), stop=(k == NK - 1))
                yield
            kv_box[0] = psv

        def emit_q(sq, h, xcs, acc_tag="po", fill=None):
            """Q projection + RoPE for head h, quarter sq. A couple of filler
            matmuls are pumped BEFORE the rope is emitted so the fillers'
            PSUM-WAR drains run ahead of the rope in the DVE queue; the rest
            cover the rope's latency before the first score matmul."""
            s0 = sq * QC
            psq = ps_tile(acc_tag, name="psq")
            for k in range(NK):
                nc.tensor.matmul(psq, lhsT=wall_t[:, k, 2 + h],
                                 rhs=xcs[k // CW][:, k % CW],
                                 start=(k == 0), stop=(k == NK - 1))
            if fill is not None:
                pump(fill, 2)
            rope_from_psum(psq, new_qt(h), s0)
            if fill is not None:
                pump(fill, 6)

        def emit_attn_head(sq, h, po_tag="po", filler=None):
            """Causal attention for head h over quarter sq's queries.

            Scores are [key-tile, q] transposed; diagonal key-tiles are
            trimmed to the columns that aren't fully masked, and the
            128-wide triangle on the diagonal gets the 0/1 mask."""
            s0 = sq * QC
            njt = 4 * (sq + 1)
            sacc = spool.tile([128, QC], dt.bfloat16, tag="sacc", name="sacc")
            po = ps_tile(po_tag, name="po")
            for jt in range(njt):
                r = jt - (njt - 4)          # >=0 -> diagonal tile index
                c0 = 128 * r if r > 0 else 0
                psc = ps_tile("sc", name="psc")
                nc.tensor.matmul(psc[:, c0:QC], lhsT=kt[:, jt * KT:(jt + 1) * KT],
                                 rhs=cur_qt[h][:, c0:QC],
                                 start=True, stop=True)
                e = epool.tile([128, QC], dt.bfloat16, tag="e", name="e")
                nc.scalar.activation(out=e[:, c0:QC], in_=psc[:, c0:QC],
                                     func=mybir.ActivationFunctionType.Exp,
                                     bias=c_bias, scale=SCALE)
                if r >= 0:
                    nc.vector.tensor_mul(e[:, c0:c0 + 128], e[:, c0:c0 + 128],
                                         c_tri)
                if jt == 0:
                    nc.vector.tensor_copy(sacc, e[:, 0:QC])
                else:
                    nc.vector.tensor_add(sacc[:, c0:QC], sacc[:, c0:QC],
                                         e[:, c0:QC])
                nc.tensor.matmul(po[:, c0:QC], lhsT=vn[:, jt], rhs=e[:, c0:QC],
                                 start=(jt == 0), stop=(jt == njt - 1))
                if filler is not None:
                    # one out-proj matmul per attention tile keeps the PE
                    # saturated while the scalar engine streams the exps
                    # (exp is ~1.5x slower per tile than the sc+pv pair).
                    # Pumped AFTER the PV: a filler matmul whose inputs
                    # aren't ready yet (e.g. a chained K/V matmul gated on a
                    # prefetch) must never sit in front of a ready PV in the
                    # in-order PE queue.
                    pump(filler, 1)
            # normalization: ctx = O * (1/colsum(E)). colsum broadcast to all
            # 128 partitions by a single ones[128,128] matmul (the all-ones
            # warmup tile), then a fast approximate reciprocal and one DVE
            # scale. Short chain, nothing on gpsimd.
            pcsb = ps_tile("sc", name="pcsb")
            nc.tensor.matmul(pcsb, lhsT=w1, rhs=sacc, start=True, stop=True)
            rec = npool.tile([128, QC], dt.float32, tag="rec", name="rec")
            nc.vector.reciprocal_approx_fast(out=rec, in_=pcsb)
            nc.vector.tensor_mul(ctxT[h][:, sq % 2, :], po, rec)

        def outproj_steps(st, pool_tag="out", dma_q="sync", dve_drains=False):
            """Generator form of the out-projection block
            out[st*128:(st+1)*128, :] = sum_h ctxT[h][:, st-block].T @ Wo[h];
            yields after each matmul so it can be pumped as PE filler inside
            the attention loop. With dve_drains, PSUM drains stay off the
            scalar engine (busy with exp) entirely."""
            half, colq = (st // 4) % 2, (st % 4) * 128
            for quad in range(4):
                ob = obp.tile([128, DIN // 4], dt.bfloat16, tag="ob", name="ob")
                for j in range(2):
                    oc = quad * 2 + j
                    tag = (pool_tag[oc % 2]
                           if isinstance(pool_tag, tuple) else pool_tag)
                    pso = ps_tile(tag, (128, 512), name="pso")
                    for h in range(HPC):
                        nc.tensor.matmul(pso,
                                         lhsT=ctxT[h][:, half, colq:colq + 128],
                                         rhs=w_o[:, h, oc * 512:(oc + 1) * 512],
                                         start=(h == 0), stop=(h == HPC - 1))
                        yield
                    # drain right away: the PSUM slot must be free well
                    # before the pumped rotation reaches it again
                    dst = ob[:, j * 512:(j + 1) * 512]
                    if dve_drains or oc % 2:
                        nc.vector.tensor_copy(dst, pso)
                    else:
                        nc.scalar.copy(dst, pso)
                eng = nc.sync if (dma_q == "sync" or quad % 2 == 0) else nc.scalar
                eng.dma_start(
                    out=out[st * 128:(st + 1) * 128,
                            quad * (DIN // 4):(quad + 1) * (DIN // 4)],
                    in_=ob)

        def emit_outproj_block(st, pool_tag="out", dma_q="sync",
                               dve_drains=False):
            for _ in outproj_steps(st, pool_tag, dma_q, dve_drains):
                pass

        _DONE = object()

        def pump(it, n):
            for _ in range(n):
                if next(it, _DONE) is _DONE:
                    return

        def prefetch_x(sq1, gs, queue="sync"):
            """Load x chunks for quarter sq1 (double-buffered: no WAR wait)."""
            eng = nc.sync if queue == "sync" else nc.scalar
            tiles = []
            for g in gs:
                xc = xp.tile([128, CW, QC], dt.bfloat16,
                             tag=f"xc{g}", name=f"xc{g}_{sq1}")
                eng.dma_start(out=xc, in_=x[:, sq1, g * CW:(g + 1) * CW])
                tiles.append(xc)
            return tiles

        # ---- quarter 0: chunk-synchronous projection waves ----
        # All six streams consume chunk w together (24 matmuls ~ 5us per
        # chunk), so the cold-start DMA keeps pace and the PE stays hot
        # from the first chunk on. In the last wave each stream's epilogue
        # (RoPE / V handoff) is emitted right after its final matmul so the
        # DVE works through the epilogues while the PE finishes the wave.
        accs = [ps_tile("proj", name="wavK"), ps_tile("proj", name="wavV"),
                ps_tile("po", name="wavQ0"), ps_tile("po", name="wavQ1"),
                ps_tile("out", name="wavQ2"), ps_tile("out", name="wavQ3")]
        vtmp0 = None
        for w in range(NXC):
            for s in range(NS):
                for kk in range(CW):
                    k = w * CW + kk
                    nc.tensor.matmul(accs[s], lhsT=wall_t[:, k, s],
                                     rhs=xcs0[w][:, kk],
                                     start=(k == 0), stop=(k == NK - 1))
                if w == NXC - 1:
                    if s == 0:
                        rope_from_psum(accs[0], kt[:, 0:QC], 0)
                    elif s == 1:
                        vtmp0 = tpool.tile([DH, QC], dt.bfloat16, tag="vtmp",
                                           name="vtmp")
                        nc.scalar.copy(vtmp0, accs[1])
                    elif s == 2:
                        rope_from_psum(accs[2], new_qt(0), 0)
            if w == 4:
                nxt = prefetch_x(1, range(0, 4), queue="sync")
            elif w == 6:
                nxt += prefetch_x(1, range(4, NXC), queue="sync")
        for i in range(QC // 128):
            pvt = ps_tile("sc", (128, 128), dt.bfloat16, name="pvt")
            nc.tensor.transpose(pvt, vtmp0[:, i * 128:(i + 1) * 128], c_id)
            if i % 2 == 0:
                nc.scalar.copy(vn[:, i], pvt)
            else:
                nc.vector.tensor_copy(vn[:, i], pvt)
        # quarter-1 K/V projections pumped as PE filler into quarter 0's
        # attention -- each head's attention alone (~1.1us) cannot cover its
        # RoPE latency (~2.1us of serial DVE), so the heads would otherwise
        # drift apart waiting on the DVE.
        fill0 = kv_steps(1, [nxt])
        rope_from_psum(accs[3], new_qt(1), 0)                  # Q1
        emit_attn_head(0, 0, po_tag="proj")
        rope_from_psum(accs[4], new_qt(2), 0)                  # Q2
        emit_attn_head(0, 1, po_tag="proj", filler=fill0)
        pump(fill0, 10)
        rope_from_psum(accs[5], new_qt(3), 0)                  # Q3
        emit_attn_head(0, 2, po_tag="po", filler=fill0)
        pump(fill0, 10)
        emit_attn_head(0, 3, po_tag="po", filler=fill0)
        for _ in fill0:
            pass
        vtmp1 = v_drain(kv_box[0])
        emit_outproj_block(0, pool_tag="out")
        v_transposes(1, vtmp1)

        # ---- quarters 1..3: stream-serial (x already resident) ----
        # The previous quarter's three inner out-proj blocks plus the NEXT
        # quarter's K/V projections are pumped as fine-grained PE filler:
        # ~8 matmuls cover each head's Q->RoPE latency, one matmul per
        # attention tile absorbs the exp lag, and the chained K/V work
        # keeps the last head's attention fed.
        xcs = nxt
        for sq in range(1, NQ):
            nxt_ref = [None]
            gens = [outproj_steps(4 * (sq - 1) + 1, "out", dve_drains=True),
                    outproj_steps(4 * (sq - 1) + 2, "proj", dve_drains=True),
                    outproj_steps(4 * (sq - 1) + 3, "out", dve_drains=True)]
            if sq + 1 < NQ:
                gens.append(kv_steps(sq + 1, nxt_ref))
            fill = itertools.chain(*gens)
            for h in range(HPC):
                emit_q(sq, h, xcs, fill=fill)
                if h == 1 and sq + 1 < NQ:
                    nxt_ref[0] = prefetch_x(sq + 1, range(0, 4), queue="sync")
                if h == 2 and sq + 1 < NQ:
                    nxt_ref[0] = nxt_ref[0] + prefetch_x(sq + 1, range(4, NXC),
                                                         queue="sync")
                emit_attn_head(sq, h, filler=fill)
                pump(fill, 8)
            for _ in fill:
                pass
            if sq + 1 < NQ:
                vtmp_n = v_drain(kv_box[0])
            # quarter-boundary block on the "sc"/"proj" arenas (free after
            # the last head's attention); DVE drains -- the scalar engine
            # still has the last head's exp backlog
            emit_outproj_block(4 * sq, pool_tag=("sc", "proj"),
                               dma_q=("alt" if sq == NQ - 1 else "sync"),
                               dve_drains=True)
            if sq + 1 < NQ:
                v_transposes(sq + 1, vtmp_n)
                xcs = nxt_ref[0]
        # ---- tail blocks: 4-deep psum rotation, alternate output queues ----
        for st, tag in ((13, ("out", "po")), (14, ("proj", "sc")),
                        (15, ("out", "po"))):
            emit_outproj_block(st, pool_tag=tag, dma_q="alt")
    nc.finalize()
    return nc


def make_in_maps(input_tensor, cos, sin, Wq, Wk, Wv, Wo):
    """Host-side sharding + layout preparation. Returns list of 8 dicts."""
    x2 = np.ascontiguousarray(input_tensor.reshape(S, DIN))
    # x_host[p, sq, k, sc] = x2[512*sq+sc, 128*k+p]
    xt = x2.T.astype(BF16)                      # [DIN, S]
    x_host = np.ascontiguousarray(
        xt.reshape(NK, 128, NQ, QC).transpose(1, 2, 0, 3))

    cos_h = np.ascontiguousarray(cos.T[0:64].astype(BF16))
    sin_h = np.ascontiguousarray(sin.T[0:64].astype(BF16))

    p_idx = np.arange(128)[:, None]
    c_idx = np.arange(128)[None, :]
    tri = (p_idx <= c_idx).astype(BF16)
    ident = np.eye(128, dtype=BF16)

    common = dict(x=x_host, cos_h=cos_h, sin_h=sin_h, tri=tri, ident=ident)

    in_maps = []
    for c in range(NCORES):
        wq_s = Wq[:, c * DPC:(c + 1) * DPC].astype(BF16)
        wq_host = wq_s.reshape(NK, 128, HPC, DH).transpose(1, 0, 2, 3)
        wk_s = Wk[:, c * DH:(c + 1) * DH].astype(BF16)
        wk_host = wk_s.reshape(NK, 128, DH).transpose(1, 0, 2)
        wv_s = Wv[:, c * DH:(c + 1) * DH].astype(BF16)
        wv_host = wv_s.reshape(NK, 128, DH).transpose(1, 0, 2)
        # wall[p, k, s, d]: s = (K, V, Q0..Q3)
        wall_host = np.ascontiguousarray(np.stack(
            [wk_host, wv_host] + [wq_host[:, :, h] for h in range(HPC)], axis=2))
        wo_s = Wo[c * DPC:(c + 1) * DPC, :].astype(BF16)
        wo_host = np.ascontiguousarray(wo_s.reshape(HPC, 128, DIN).transpose(1, 0, 2))
        in_maps.append(dict(common, wall=wall_host, wo=wo_host))
    return in_maps


def _numpy_fallback(input_tensor, attention_mask, cos, sin, Wq, Wk, Wv, Wo):
    x = input_tensor.astype(np.float32)
    b, s, _ = x.shape
    q = (x @ Wq).reshape(b, s, H, DH).transpose(0, 2, 1, 3)
    k = (x @ Wk).reshape(b, s, KV, DH).transpose(0, 2, 1, 3)
    v = (x @ Wv).reshape(b, s, KV, DH).transpose(0, 2, 1, 3)

    def rope(t):
        t1, t2 = t[..., :64], t[..., 64:]
        rot = np.concatenate([-t2, t1], axis=-1)
        return t * cos[None, None] + rot * sin[None, None]

    q, k = rope(q), rope(k)
    k = np.repeat(k, G, axis=1)
    v = np.repeat(v, G, axis=1)
    sc = np.einsum('bhqd,bhkd->bhqk', q, k)
    sc = np.where(attention_mask, -np.inf, sc) / np.float32(np.sqrt(DH))
    sc = sc - sc.max(axis=-1, keepdims=True)
    w = np.exp(sc)
    w = w / w.sum(axis=-1, keepdims=True)
    ctx = np.einsum('bhqk,bhkd->bhqd', w, v)
    ctx = ctx.transpose(0, 2, 1, 3).reshape(b, s, H * DH)
    return (ctx @ Wo).astype(np.float32)


_NC_CACHE = {}


def kernel(input_tensor, attention_mask, cos, sin, Wq, Wk, Wv, Wo):
    mask = np.asarray(attention_mask).reshape(S, S)
    causal = np.array_equal(mask, np.triu(np.ones((S, S), bool), k=1))
    cos_a, sin_a = np.asarray(cos), np.asarray(sin)
    halved = (np.array_equal(cos_a[:, :64], cos_a[:, 64:])
              and np.array_equal(sin_a[:, :64], sin_a[:, 64:]))
    if not (causal and halved):
        return _numpy_fallback(np.asarray(input_tensor), np.asarray(attention_mask),
                               np.asarray(cos), np.asarray(sin),
                               np.asarray(Wq), np.asarray(Wk),
                               np.asarray(Wv), np.asarray(Wo))

    if "nc" not in _NC_CACHE:
        _NC_CACHE["nc"] = build_nc()
    nc = _NC_CACHE["nc"]

    in_maps = make_in_maps(np.asarray(input_tensor), np.asarray(cos),
                           np.asarray(sin), np.asarray(Wq), np.asarray(Wk),
                           np.asarray(Wv), np.asarray(Wo))
    res = run_bass_kernel_spmd(nc, in_maps, core_ids=list(range(NCORES)))
    acc = np.zeros((S, DIN), np.float32)
    for r in res.results:
        acc += np.asarray(r["out"], dtype=np.float32)
    return acc.reshape(1, S, DIN)


# revision 43
# speedup vs baseline: 1.1789x; 1.0176x over previous
"""GroupQueryAttention on 8 TRN2 NeuronCores.

Strategy: tensor-parallel over heads. H=32 query heads, KV=8 kv heads,
group size G=4 -> each core owns exactly 1 kv head and its 4 query heads.
Per core:
  - QKV projections from a replicated (pre-transposed, channels-major) input
  - RoPE on Q/K (rotate-half, done on DVE across partition halves)
  - attention with scores computed TRANSPOSED ([keys, q] layout) so the
    exp(scores) tiles feed the V-matmul directly as the moving operand;
    softmax normalization is deferred: O = V.E, then ctx = O * (1/colsum(E))
  - partial output ctx @ Wo_shard  (row-shard of Wo)
Host sums the 8 partial outputs (the "all-reduce" of the row-parallel Wo).

Perf structure (v2):
  - dual DMA queues: loads are split between the SP (sync) and Activation
    (scalar) hardware DGE queues -- the single-queue serialization of v1
    starved the PE at kernel start and backed up the output at the end
  - quarter-0 projections are emitted as a chunk-major WAVE across the six
    projection streams (K, V, Q0..Q3, weights interleaved per k-group in
    one "wall" tensor) so the PE consumes each arriving x chunk at ~1/6th
    the single-stream rate -- the cold-start DMA can keep up and the PE
    never idles (idle triggers a ~7us half-speed HAM window)
  - x chunks double-buffered (bufs=2) so next-quarter prefetch has no WAR
    dependency and can be issued early on either queue
  - softmax normalization: colsum is computed pre-broadcast by a single
    ones[128,128] matmul (same cost as the old ones-column matmul), then
    DVE reciprocal_approx_fast + DVE scale. No gpsimd hop (the
    partition_broadcast custom op had ~1.2us latency on the critical path)
  - out-projection PSUM allocations rotate across three pool tags so a
    block's matmuls never wait on drains queued behind the previous head's
    exp avalanche; tail blocks' output DMAs alternate queues
  - a warm-up chain of tiny matmuls (computing the exp bias constant)
    lifts the PE HAM clock gate during the initial DMA wait
"""

import itertools
import sys

sys.path.insert(0, "/opt/trn_rl_repo")

from contextlib import ExitStack

import numpy as np
import ml_dtypes

import concourse.bass as bass
import concourse.bacc as bacc
import concourse.tile as tile
from concourse import mybir
from concourse.bass_utils import run_bass_kernel_spmd

BF16 = ml_dtypes.bfloat16

S = 2048          # sequence length
DIN = 4096        # model dim
H, KV, DH = 32, 8, 128
G = H // KV       # 4 query heads per kv head
NCORES = 8
HPC = H // NCORES     # 4 query heads per core
DPC = HPC * DH        # 512 = per-core q-projection width

NQ = 4            # s-quarters (chunks of 512 queries)
QC = S // NQ      # 512
KT = 128          # key tile (partition dim of transposed scores)
NKT = S // KT     # 16 key tiles
NK = DIN // 128   # 32 contraction tiles for projections
NXC = 8           # x chunks per quarter (k-groups of CW)
CW = NK // NXC    # 4 k-tiles per x chunk
NS = 6            # projection streams: K, V, Q0..Q3
SCALE = 1.0 / float(np.sqrt(DH))
EXP_BIAS = -10.0  # constant shift inside exp; cancels in normalization
NWARM = 80


def build_nc():
    """Build the per-core Bass program (same program on all 8 cores; the
    per-core weight shards arrive via in_maps)."""
    nc = bacc.Bacc()
    dt = mybir.dt

    # ---- DRAM parameters (host-prepared layouts; all DMA-contiguous) ----
    # x[p, sq, k, sc] = x_orig[512*sq + sc, 128*k + p]   (channels-major)
    x = nc.declare_dram_parameter("x", [128, NQ, NK, QC], dt.bfloat16, isOutput=False)
    # wall[p, k, s, d]: s=0 -> Wk_shard, s=1 -> Wv_shard, s=2+h -> Wq head h
    wall = nc.declare_dram_parameter("wall", [128, NK, NS, DH], dt.bfloat16,
                                     isOutput=False)
    # wo[p, h, n] = Wo_shard[128*h + p, n]
    wo = nc.declare_dram_parameter("wo", [128, HPC, DIN], dt.bfloat16, isOutput=False)
    # RoPE half tables: rows d<64 of cos/sin transposed (rows 64-127 are
    # identical by the rotate-half construction; duplicated on-chip)
    cos_h = nc.declare_dram_parameter("cos_h", [DH // 2, S], dt.bfloat16, isOutput=False)
    sin_h = nc.declare_dram_parameter("sin_h", [DH // 2, S], dt.bfloat16, isOutput=False)
    # tri[p, c] = 1.0 if p <= c else 0.0  (128x128 causal triangle)
    tri = nc.declare_dram_parameter("tri", [128, 128], dt.bfloat16, isOutput=False)
    ident = nc.declare_dram_parameter("ident", [128, 128], dt.bfloat16, isOutput=False)
    out = nc.declare_dram_parameter("out", [S, DIN], dt.bfloat16, isOutput=True)

    with tile.TileContext(nc) as tc, ExitStack() as ctx:
        singles = ctx.enter_context(tc.tile_pool(name="singles", bufs=1))
        wop = ctx.enter_context(tc.tile_pool(name="wop", bufs=1))
        xp = ctx.enter_context(tc.tile_pool(name="xp", bufs=2))
        qkv = ctx.enter_context(tc.tile_pool(name="qkv", bufs=1))
        epool = ctx.enter_context(tc.tile_pool(name="epool", bufs=5))
        spool = ctx.enter_context(tc.tile_pool(name="spool", bufs=3))
        npool = ctx.enter_context(tc.tile_pool(name="npool", bufs=2))
        tpool = ctx.enter_context(tc.tile_pool(name="tpool", bufs=2))
        obp = ctx.enter_context(tc.tile_pool(name="obp", bufs=3))
        # One PSUM pool, four 2-slot tag arenas = all 8 banks.
        psum = ctx.enter_context(tc.tile_pool(name="psum", bufs=2, space="PSUM"))

        def ps_tile(tag, shape=(128, QC), dtyp=dt.float32, name="ps"):
            return psum.tile(list(shape), dtyp, tag=tag, name=name)

        # ---- SBUF residents ----
        wall_t = singles.tile([128, NK, NS, DH], dt.bfloat16, tag="wall")
        c_cos = singles.tile([DH, S], dt.bfloat16, tag="cos")
        c_sin = singles.tile([DH, S], dt.bfloat16, tag="sin")
        c_tri = singles.tile([128, 128], dt.bfloat16, tag="tri")
        c_id = singles.tile([128, 128], dt.bfloat16, tag="ident")
        w_o = wop.tile([128, HPC, DIN], dt.bfloat16, tag="wo")

        xcs0 = [xp.tile([128, CW, QC], dt.bfloat16, tag=f"xc{g}",
                        name=f"xc{g}_0") for g in range(NXC)]

        # ---- early DMA schedule on two queues, in consumption order ----
        # Quarter-0 runs chunk-synchronous waves (all six streams consume
        # chunk w together, ~5us per chunk), so deliveries need only keep
        # that pace. sync (SP) queue: the x chunks + wall group 1;
        # scalar (Activation) queue: the remaining wall groups, cos/sin, wo.
        def wall_grp(g, s0_, s1_, eng):
            eng.dma_start(out=wall_t[:, g * CW:(g + 1) * CW, s0_:s1_],
                          in_=wall[:, g * CW:(g + 1) * CW, s0_:s1_])

        # every wall group is split (a = K,V; b = Q0..Q3): the wave's K/V
        # matmuls gate on the small leading a-piece instead of the whole
        # 768KB group (the unsplit form showed a 4.8us LDWEIGHTS wait)
        wall_grp(0, 0, 2, nc.scalar)
        nc.sync.dma_start(out=xcs0[0], in_=x[:, 0, 0:CW])
        wall_grp(0, 2, NS, nc.scalar)
        nc.sync.dma_start(out=xcs0[1], in_=x[:, 0, CW:2 * CW])
        wall_grp(2, 0, 2, nc.scalar)
        wall_grp(1, 0, 2, nc.sync)
        wall_grp(1, 2, NS, nc.sync)
        wall_grp(2, 2, NS, nc.scalar)
        wall_grp(3, 0, 2, nc.scalar)
        wall_grp(3, 2, NS, nc.scalar)
        for g in range(2, NXC):
            nc.sync.dma_start(out=xcs0[g], in_=x[:, 0, g * CW:(g + 1) * CW])
            if g == 3:
                wall_grp(4, 0, 2, nc.scalar)
                wall_grp(4, 2, NS, nc.scalar)
            elif g == 4:
                nc.scalar.dma_start(out=c_cos[0:64], in_=cos_h[:])
                nc.scalar.dma_start(out=c_sin[64:128], in_=sin_h[:])
            elif g >= 5:
                wall_grp(g, 0, 2, nc.scalar)
                wall_grp(g, 2, NS, nc.scalar)
        nc.sync.dma_start(out=c_tri, in_=tri[:])
        nc.sync.dma_start(out=c_id, in_=ident[:])
        for h in range(HPC):
            nc.scalar.dma_start(out=w_o[:, h], in_=wo[:, h])

        # ---- PE warm-up + exp-bias constant ----
        # A serial chain of tiny matmuls during the initial DMA wait keeps
        # the PE busy (the dependency chain paces it at latency rate, which
        # is the point: coverage, not throughput) so the HAM clock-gate
        # releases before the first real matmul. The chain accumulates zeros
        # and finally ones^T @ (EXP_BIAS/128), producing the exp bias
        # vector -- a live chain, so nothing is DCE'd.
        w1 = singles.tile([128, 128], dt.bfloat16, tag="warm1")
        nc.vector.memset(w1, 1.0)
        wz = singles.tile([128, 1], dt.bfloat16, tag="warmz")
        nc.vector.memset(wz, 0.0)
        wb = singles.tile([128, 1], dt.bfloat16, tag="warmb")
        nc.vector.memset(wb, EXP_BIAS / 128.0)
        ps_warm = ps_tile("sc", (128, 1), name="ps_warm")
        for i in range(NWARM):
            nc.tensor.matmul(ps_warm, lhsT=w1,
                             rhs=(wz if i < NWARM - 1 else wb),
                             start=(i == 0), stop=(i == NWARM - 1))
        c_bias = singles.tile([128, 1], dt.float32, tag="ebias")
        nc.scalar.copy(c_bias, ps_warm)

        # derive full RoPE tables (upper cos half copy; lower sin half negate)
        nc.vector.tensor_copy(c_cos[64:128], c_cos[0:64])
        nc.vector.tensor_scalar_mul(c_sin[0:64], c_sin[64:128], -1.0)

        # ---- long-lived activations ----
        # qt: one [DH, QC] buffer per head, rewritten each quarter.
        # ctxT: two quarters per head (blocks of quarter sq are consumed
        # during quarter sq+1 while sq+1's attention writes the other half).
        kt = qkv.tile([DH, S], dt.bfloat16, tag="kt")
        vn = qkv.tile([128, NKT, DH], dt.bfloat16, tag="vn")   # V natural tiles
        ctxT = [qkv.tile([DH, 2, QC], dt.bfloat16, tag=f"ctx{h}", name=f"ctx{h}")
                for h in range(HPC)]
        cur_qt = [None] * HPC

        def new_qt(h):
            t = qkv.tile([DH, QC], dt.bfloat16, tag=f"qt{h}", name=f"qt{h}")
            cur_qt[h] = t
            return t

        def rope_from_psum(ps, dst_slice, s0):
            """dst = ps*cos + rot_half(ps)*sinm over s-columns [s0, s0+QC)."""
            t1 = tpool.tile([DH, QC], dt.float32, tag="t1", name="t1")
            nc.vector.tensor_mul(t1, ps, c_cos[:, s0:s0 + QC])
            t2 = tpool.tile([DH, QC], dt.float32, tag="t2", name="t2")
            nc.vector.tensor_mul(t2[0:64, :], ps[64:128, :], c_sin[0:64, s0:s0 + QC])
            nc.vector.tensor_mul(t2[64:128, :], ps[0:64, :], c_sin[64:128, s0:s0 + QC])
            nc.vector.tensor_add(dst_slice, t1, t2)

        def v_drain(psv):
            """V PSUM -> bf16 (scalar engine), releasing the PSUM slot."""
            vtmp = tpool.tile([DH, QC], dt.bfloat16, tag="vtmp", name="vtmp")
            nc.scalar.copy(vtmp, psv)
            return vtmp

        def v_transposes(sq, vtmp):
            """128x128 transposes into vn. The pvt->vn copies alternate
            between the scalar engine and the DVE so neither the upcoming
            exp stream (scalar) nor the RoPE chain (DVE) is pushed back by
            the full set."""
            for i in range(QC // 128):
                pvt = ps_tile("sc", (128, 128), dt.bfloat16, name="pvt")
                nc.tensor.transpose(pvt, vtmp[:, i * 128:(i + 1) * 128], c_id)
                if i % 2 == 0:
                    nc.scalar.copy(vn[:, sq * 4 + i], pvt)
                else:
                    nc.vector.tensor_copy(vn[:, sq * 4 + i], pvt)

        kv_box = [None]   # psv of the pumped next-quarter V projection

        def kv_steps(sq2, xref):
            """Generator: the NEXT quarter's K and V projections, pumped as
            PE filler into the current quarter's attention. RoPE(K) is
            emitted inline; the V transposes are left to the caller (their
            pvt slots and vn copies would collide with the exp stream).
            xref is a 1-element box so prefetched chunks can be supplied
            after the generator is constructed."""
            s02 = sq2 * QC
            psk = ps_tile("proj", name="psk")
            for k in range(NK):
                nc.tensor.matmul(psk, lhsT=wall_t[:, k, 0],
                                 rhs=xref[0][k // CW][:, k % CW],
                                 start=(k == 0), stop=(k == NK - 1))
                yield
            rope_from_psum(psk, kt[:, s02:s02 + QC], s02)
            psv = ps_tile("proj", name="psv")
            for k in range(NK):
                nc.tensor.matmul(psv, lhsT=wall_t[:, k, 1],
                                 rhs=xref[0][k // CW][:, k % CW],
                                 start=(k == 0), stop=(k == NK - 1))
                yield
            kv_box[0] = psv

        def emit_q(sq, h, xcs, acc_tag="po", fill=None):
            """Q projection + RoPE for head h, quarter sq. A couple of filler
            matmuls are pumped BEFORE the rope is emitted so the fillers'
            PSUM-WAR drains run ahead of the rope in the DVE queue; the rest
            cover the rope's latency before the first score matmul."""
            s0 = sq * QC
            psq = ps_tile(acc_tag, name="psq")
            for k in range(NK):
                nc.tensor.matmul(psq, lhsT=wall_t[:, k, 2 + h],
                                 rhs=xcs[k // CW][:, k % CW],
                                 start=(k == 0), stop=(k == NK - 1))
            if fill is not None:
                pump(fill, 2)
            rope_from_psum(psq, new_qt(h), s0)
            if fill is not None:
                pump(fill, 6)

        def emit_attn_head(sq, h, po_tag="po", filler=None):
            """Causal attention for head h over quarter sq's queries.

            Scores are [key-tile, q] transposed; diagonal key-tiles are
            trimmed to the columns that aren't fully masked, and the
            128-wide triangle on the diagonal gets the 0/1 mask."""
            s0 = sq * QC
            njt = 4 * (sq + 1)
            sacc = spool.tile([128, QC], dt.bfloat16, tag="sacc", name="sacc")
            po = ps_tile(po_tag, name="po")
            for jt in range(njt):
                r = jt - (njt - 4)          # >=0 -> diagonal tile index
                c0 = 128 * r if r > 0 else 0
                psc = ps_tile("sc", name="psc")
                nc.tensor.matmul(psc[:, c0:QC], lhsT=kt[:, jt * KT:(jt + 1) * KT],
                                 rhs=cur_qt[h][:, c0:QC],
                                 start=True, stop=True)
                e = epool.tile([128, QC], dt.bfloat16, tag="e", name="e")
                nc.scalar.activation(out=e[:, c0:QC], in_=psc[:, c0:QC],
                                     func=mybir.ActivationFunctionType.Exp,
                                     bias=c_bias, scale=SCALE)
                if r >= 0:
                    nc.vector.tensor_mul(e[:, c0:c0 + 128], e[:, c0:c0 + 128],
                                         c_tri)
                if jt == 0:
                    nc.vector.tensor_copy(sacc, e[:, 0:QC])
                else:
                    nc.vector.tensor_add(sacc[:, c0:QC], sacc[:, c0:QC],
                                         e[:, c0:QC])
                nc.tensor.matmul(po[:, c0:QC], lhsT=vn[:, jt], rhs=e[:, c0:QC],
                                 start=(jt == 0), stop=(jt == njt - 1))
                if filler is not None:
                    # one out-proj matmul per attention tile keeps the PE
                    # saturated while the scalar engine streams the exps
                    # (exp is ~1.5x slower per tile than the sc+pv pair).
                    # Pumped AFTER the PV: a filler matmul whose inputs
                    # aren't ready yet (e.g. a chained K/V matmul gated on a
                    # prefetch) must never sit in front of a ready PV in the
                    # in-order PE queue.
                    pump(filler, 1)
            # normalization: ctx = O * (1/colsum(E)). colsum broadcast to all
            # 128 partitions by a single ones[128,128] matmul (the all-ones
            # warmup tile), then a fast approximate reciprocal and one DVE
            # scale. Short chain, nothing on gpsimd.
            pcsb = ps_tile("sc", name="pcsb")
            nc.tensor.matmul(pcsb, lhsT=w1, rhs=sacc, start=True, stop=True)
            rec = npool.tile([128, QC], dt.float32, tag="rec", name="rec")
            nc.vector.reciprocal_approx_fast(out=rec, in_=pcsb)
            nc.vector.tensor_mul(ctxT[h][:, sq % 2, :], po, rec)

        def outproj_steps(st, pool_tag="out", dma_q="sync", dve_drains=False):
            """Generator form of the out-projection block
            out[st*128:(st+1)*128, :] = sum_h ctxT[h][:, st-block].T @ Wo[h];
            yields after each matmul so it can be pumped as PE filler inside
            the attention loop. With dve_drains, PSUM drains stay off the
            scalar engine (busy with exp) entirely."""
            half, colq = (st // 4) % 2, (st % 4) * 128
            for quad in range(4):
                ob = obp.tile([128, DIN // 4], dt.bfloat16, tag="ob", name="ob")
                for j in range(2):
                    oc = quad * 2 + j
                    tag = (pool_tag[oc % 2]
                           if isinstance(pool_tag, tuple) else pool_tag)
                    pso = ps_tile(tag, (128, 512), name="pso")
                    for h in range(HPC):
                        nc.tensor.matmul(pso,
                                         lhsT=ctxT[h][:, half, colq:colq + 128],
                                         rhs=w_o[:, h, oc * 512:(oc + 1) * 512],
                                         start=(h == 0), stop=(h == HPC - 1))
                        yield
                    # drain right away: the PSUM slot must be free well
                    # before the pumped rotation reaches it again
                    dst = ob[:, j * 512:(j + 1) * 512]
                    if dve_drains or oc % 2:
                        nc.vector.tensor_copy(dst, pso)
                    else:
                        nc.scalar.copy(dst, pso)
                eng = nc.sync if (dma_q == "sync" or quad % 2 == 0) else nc.scalar
                eng.dma_start(
                    out=out[st * 128:(st + 1) * 128,
                            quad * (DIN // 4):(quad + 1) * (DIN // 4)],
                    in_=ob)

        def emit_outproj_block(st, pool_tag="out", dma_q="sync",
                               dve_drains=False):
            for _ in outproj_steps(st, pool_tag, dma_q, dve_drains):
                pass

        _DONE = object()

        def pump(it, n):
            for _ in range(n):
                if next(it, _DONE) is _DONE:
                    return

        def prefetch_x(sq1, gs, queue="sync"):
            """Load x chunks for quarter sq1 (double-buffered: no WAR wait)."""
            eng = nc.sync if queue == "sync" else nc.scalar
            tiles = []
            for g in gs:
                xc = xp.tile([128, CW, QC], dt.bfloat16,
                             tag=f"xc{g}", name=f"xc{g}_{sq1}")
                eng.dma_start(out=xc, in_=x[:, sq1, g * CW:(g + 1) * CW])
                tiles.append(xc)
            return tiles

        # ---- quarter 0: chunk-synchronous projection waves ----
        # All six streams consume chunk w together (24 matmuls ~ 5us per
        # chunk), so the cold-start DMA keeps pace and the PE stays hot
        # from the first chunk on. In the last wave each stream's epilogue
        # (RoPE / V handoff) is emitted right after its final matmul so the
        # DVE works through the epilogues while the PE finishes the wave.
        accs = [ps_tile("proj", name="wavK"), ps_tile("proj", name="wavV"),
                ps_tile("po", name="wavQ0"), ps_tile("po", name="wavQ1"),
                ps_tile("out", name="wavQ2"), ps_tile("out", name="wavQ3")]
        vtmp0 = None
        for w in range(NXC):
            for s in range(NS):
                for kk in range(CW):
                    k = w * CW + kk
                    nc.tensor.matmul(accs[s], lhsT=wall_t[:, k, s],
                                     rhs=xcs0[w][:, kk],
                                     start=(k == 0), stop=(k == NK - 1))
                if w == NXC - 1:
                    if s == 0:
                        rope_from_psum(accs[0], kt[:, 0:QC], 0)
                    elif s == 1:
                        vtmp0 = tpool.tile([DH, QC], dt.bfloat16, tag="vtmp",
                                           name="vtmp")
                        nc.scalar.copy(vtmp0, accs[1])
                    elif s == 2:
                        rope_from_psum(accs[2], new_qt(0), 0)
            if w == 4:
                nxt = prefetch_x(1, range(0, 4), queue="sync")
            elif w == 6:
                nxt += prefetch_x(1, range(4, NXC), queue="sync")
        for i in range(QC // 128):
            pvt = ps_tile("sc", (128, 128), dt.bfloat16, name="pvt")
            nc.tensor.transpose(pvt, vtmp0[:, i * 128:(i + 1) * 128], c_id)
            if i % 2 == 0:
                nc.scalar.copy(vn[:, i], pvt)
            else:
                nc.vector.tensor_copy(vn[:, i], pvt)
        # quarter-1 K/V projections pumped as PE filler into quarter 0's
        # attention -- each head's attention alone (~1.1us) cannot cover its
        # RoPE latency (~2.1us of serial DVE), so the heads would otherwise
        # drift apart waiting on the DVE.
        fill0 = kv_steps(1, [nxt])
        rope_from_psum(accs[3], new_qt(1), 0)                  # Q1
        emit_attn_head(0, 0, po_tag="proj")
        rope_from_psum(accs[4], new_qt(2), 0)                  # Q2
        emit_attn_head(0, 1, po_tag="proj", filler=fill0)
        pump(fill0, 10)
        rope_from_psum(accs[5], new_qt(3), 0)                  # Q3
        emit_attn_head(0, 2, po_tag="po", filler=fill0)
        pump(fill0, 10)
        emit_attn_head(0, 3, po_tag="po", filler=fill0)
        for _ in fill0:
            pass
        vtmp1 = v_drain(kv_box[0])
        emit_outproj_block(0, pool_tag="out")
        v_transposes(1, vtmp1)

        # ---- quarters 1..3: stream-serial (x already resident) ----
        # The previous quarter's three inner out-proj blocks plus the NEXT
        # quarter's K/V projections are pumped as fine-grained PE filler:
        # ~8 matmuls cover each head's Q->RoPE latency, one matmul per
        # attention tile absorbs the exp lag, and the chained K/V work
        # keeps the last head's attention fed.
        xcs = nxt
        for sq in range(1, NQ):
            nxt_ref = [None]
            gens = [outproj_steps(4 * (sq - 1) + 1, "out", dve_drains=True),
                    outproj_steps(4 * (sq - 1) + 2, "proj", dve_drains=True),
                    outproj_steps(4 * (sq - 1) + 3, "out", dve_drains=True)]
            if sq + 1 < NQ:
                gens.append(kv_steps(sq + 1, nxt_ref))
            fill = itertools.chain(*gens)
            for h in range(HPC):
                emit_q(sq, h, xcs, fill=fill)
                if h == 1 and sq + 1 < NQ:
                    nxt_ref[0] = prefetch_x(sq + 1, range(0, 4), queue="sync")
                if h == 2 and sq + 1 < NQ:
                    nxt_ref[0] = nxt_ref[0] + prefetch_x(sq + 1, range(4, NXC),
                                                         queue="sync")
                emit_attn_head(sq, h, filler=fill)
                pump(fill, 8)
            for _ in fill:
                pass
            if sq + 1 < NQ:
                vtmp_n = v_drain(kv_box[0])
            # quarter-boundary block on the "sc"/"proj" arenas (free after
            # the last head's attention); DVE drains -- the scalar engine
            # still has the last head's exp backlog
            emit_outproj_block(4 * sq, pool_tag=("sc", "proj"),
                               dma_q=("alt" if sq == NQ - 1 else "sync"),
                               dve_drains=True)
            if sq + 1 < NQ:
                v_transposes(sq + 1, vtmp_n)
                xcs = nxt_ref[0]
        # ---- tail blocks: 4-deep psum rotation, alternate output queues ----
        for st, tag in ((13, ("out", "po")), (14, ("proj", "sc")),
                        (15, ("out", "po"))):
            emit_outproj_block(st, pool_tag=tag, dma_q="alt")
    nc.finalize()
    return nc


def make_in_maps(input_tensor, cos, sin, Wq, Wk, Wv, Wo):
    """Host-side sharding + layout preparation. Returns list of 8 dicts."""
    x2 = np.ascontiguousarray(input_tensor.reshape(S, DIN))
    # x_host[p, sq, k, sc] = x2[512*sq+sc, 128*k+p]
    xt = x2.T.astype(BF16)                      # [DIN, S]
    x_host = np.ascontiguousarray(
        xt.reshape(NK, 128, NQ, QC).transpose(1, 2, 0, 3))

    cos_h = np.ascontiguousarray(cos.T[0:64].astype(BF16))
    sin_h = np.ascontiguousarray(sin.T[0:64].astype(BF16))

    p_idx = np.arange(128)[:, None]
    c_idx = np.arange(128)[None, :]
    tri = (p_idx <= c_idx).astype(BF16)
    ident = np.eye(128, dtype=BF16)

    common = dict(x=x_host, cos_h=cos_h, sin_h=sin_h, tri=tri, ident=ident)

    in_maps = []
    for c in range(NCORES):
        wq_s = Wq[:, c * DPC:(c + 1) * DPC].astype(BF16)
        wq_host = wq_s.reshape(NK, 128, HPC, DH).transpose(1, 0, 2, 3)
        wk_s = Wk[:, c * DH:(c + 1) * DH].astype(BF16)
        wk_host = wk_s.reshape(NK, 128, DH).transpose(1, 0, 2)
        wv_s = Wv[:, c * DH:(c + 1) * DH].astype(BF16)
        wv_host = wv_s.reshape(NK, 128, DH).transpose(1, 0, 2)
        # wall[p, k, s, d]: s = (K, V, Q0..Q3)
        wall_host = np.ascontiguousarray(np.stack(
            [wk_host, wv_host] + [wq_host[:, :, h] for h in range(HPC)], axis=2))
        wo_s = Wo[c * DPC:(c + 1) * DPC, :].astype(BF16)
        wo_host = np.ascontiguousarray(wo_s.reshape(HPC, 128, DIN).transpose(1, 0, 2))
        in_maps.append(dict(common, wall=wall_host, wo=wo_host))
    return in_maps


def _numpy_fallback(input_tensor, attention_mask, cos, sin, Wq, Wk, Wv, Wo):
    x = input_tensor.astype(np.float32)
    b, s, _ = x.shape
    q = (x @ Wq).reshape(b, s, H, DH).transpose(0, 2, 1, 3)
    k = (x @ Wk).reshape(b, s, KV, DH).transpose(0, 2, 1, 3)
    v = (x @ Wv).reshape(b, s, KV, DH).transpose(0, 2, 1, 3)

    def rope(t):
        t1, t2 = t[..., :64], t[..., 64:]
        rot = np.concatenate([-t2, t1], axis=-1)
        return t * cos[None, None] + rot * sin[None, None]

    q, k = rope(q), rope(k)
    k = np.repeat(k, G, axis=1)
    v = np.repeat(v, G, axis=1)
    sc = np.einsum('bhqd,bhkd->bhqk', q, k)
    sc = np.where(attention_mask, -np.inf, sc) / np.float32(np.sqrt(DH))
    sc = sc - sc.max(axis=-1, keepdims=True)
    w = np.exp(sc)
    w = w / w.sum(axis=-1, keepdims=True)
    ctx = np.einsum('bhqk,bhkd->bhqd', w, v)
    ctx = ctx.transpose(0, 2, 1, 3).reshape(b, s, H * DH)
    return (ctx @ Wo).astype(np.float32)


_NC_CACHE = {}


def kernel(input_tensor, attention_mask, cos, sin, Wq, Wk, Wv, Wo):
    mask = np.asarray(attention_mask).reshape(S, S)
    causal = np.array_equal(mask, np.triu(np.ones((S, S), bool), k=1))
    cos_a, sin_a = np.asarray(cos), np.asarray(sin)
    halved = (np.array_equal(cos_a[:, :64], cos_a[:, 64:])
              and np.array_equal(sin_a[:, :64], sin_a[:, 64:]))
    if not (causal and halved):
        return _numpy_fallback(np.asarray(input_tensor), np.asarray(attention_mask),
                               np.asarray(cos), np.asarray(sin),
                               np.asarray(Wq), np.asarray(Wk),
                               np.asarray(Wv), np.asarray(Wo))

    if "nc" not in _NC_CACHE:
        _NC_CACHE["nc"] = build_nc()
    nc = _NC_CACHE["nc"]

    in_maps = make_in_maps(np.asarray(input_tensor), np.asarray(cos),
                           np.asarray(sin), np.asarray(Wq), np.asarray(Wk),
                           np.asarray(Wv), np.asarray(Wo))
    res = run_bass_kernel_spmd(nc, in_maps, core_ids=list(range(NCORES)))
    acc = np.zeros((S, DIN), np.float32)
    for r in res.results:
        acc += np.asarray(r["out"], dtype=np.float32)
    return acc.reshape(1, S, DIN)
